# revision 1
# baseline (speedup 1.0000x reference)
"""Trainium2 Bass kernel for the Neural-CDE-style cell (nn_JaCDE_88167088653055).

Math (per batch row b):
    x    = spline(coeffs, t)   xdot = spline(dcoeffs, t)
    l1   = x @ wx.T + h @ wh.T + b0
    relu = relu(l1);  drelu = sigmoid(l1)
    lout = relu @ wout.T + b1; th = tanh(lout); dth = 1 - th^2
    J(v) = dth * ((drelu * v) @ wout.T)        # action of the Jacobian factor
    jx   = J(xdot @ wx.T); jxh = J(jx @ wh.T); jxhh = J(jxh @ wh.T)
    out  = jx + jxh + jxhh

Device-side reformulation:
  * the [B,H,H] d_outer tensor is never materialized; every einsum with it
    collapses to per-row elementwise multiplies around small matmuls.
  * the cubic-spline evaluation folds into the wx matmul: with
    powers = dt**[0..3],  x @ wx.T == csel_flat @ (wx (x) powers).T  where
    csel_flat = coeffs[:, idx].reshape(B, CIN*4) — so the spline costs zero
    extra device passes and the contraction is K=256.
  * tanh is computed through sigmoid (tanh(x) = 2*sigmoid(2x)-1,
    1-tanh^2 = 4*s*(1-s)) so every scalar-engine activation (Relu, Sigmoid)
    lives in one ACT table set — no per-chunk activation-table reloads.
  * m1+m2+m3 accumulate in one PSUM bank via the PE (start/stop flags), so the
    final sum costs a single vector op.

Sharding: pure data parallel — batch 8192 split as 1024 rows per core across
8 cores; the small weights are replicated. All activations live
feature-major ([feature<=128 partitions, batch free]) so every matmul is
`out.T = W @ act.T` with the contraction on partitions.
"""

import numpy as np

import concourse.bass as bass
import concourse.mybir as mybir
import concourse.tile as tile
from concourse import bacc, bass_utils

N_CORES = 8
B = 8192
NOBS = 16
CIN = 64
H = 128
K4 = CIN * 4            # 256: folded (channel, power) contraction dim
BS = B // N_CORES       # 1024 batch rows per core
CHUNK = 512             # batch columns per pipeline step (one PSUM bank)
NCH = BS // CHUNK
F32 = mybir.dt.float32
F32R = mybir.dt.float32r

USE_F32R = True         # full-rate PE path; set False for exact fp32 matmuls

_NC_CACHE = {}


def _build_nc(use_f32r: bool):
    AF = mybir.ActivationFunctionType
    OP = mybir.AluOpType

    nc = bacc.Bacc("TRN2", target_bir_lowering=False, debug=False,
                   enable_asserts=False, num_devices=N_CORES)

    # dtype of everything that feeds the PE: the BIR verifier requires every
    # producer of an fp32r matmul operand to emit fp32r-typed (rounded) data.
    MMDT = F32R if use_f32r else F32

    ct = nc.dram_tensor("ct", [K4, BS], MMDT, kind="ExternalInput")
    dct = nc.dram_tensor("dct", [K4, BS], MMDT, kind="ExternalInput")
    ht = nc.dram_tensor("ht", [H, BS], MMDT, kind="ExternalInput")
    wxpt = nc.dram_tensor("wxpt", [K4, H], MMDT, kind="ExternalInput")
    wht = nc.dram_tensor("wht", [H, H], MMDT, kind="ExternalInput")
    woutt = nc.dram_tensor("woutt", [H, H], MMDT, kind="ExternalInput")
    b0c = nc.dram_tensor("b0c", [H, 1], F32, kind="ExternalInput")
    b1c2 = nc.dram_tensor("b1c2", [H, 1], F32, kind="ExternalInput")
    outt = nc.dram_tensor("outt", [H, BS], F32, kind="ExternalOutput")

    def mm(out_ap, lhsT, rhs, start=True, stop=True):
        nc.tensor.matmul(out_ap, lhsT, rhs, start=start, stop=stop,
                         skip_group_check=True)

    with tile.TileContext(nc) as tc:
        with tc.tile_pool(name="w", bufs=1) as wp, \
             tc.tile_pool(name="io", bufs=2) as io, \
             tc.tile_pool(name="tmp", bufs=2) as tmp, \
             tc.tile_pool(name="ps", bufs=1, space="PSUM") as ps:

            wxp0 = wp.tile([128, H], MMDT, tag="wxp0")
            nc.sync.dma_start(wxp0[:], wxpt[0:128, :])
            wxp1 = wp.tile([128, H], MMDT, tag="wxp1")
            nc.sync.dma_start(wxp1[:], wxpt[128:256, :])
            whs = wp.tile([H, H], MMDT, tag="whs")
            nc.sync.dma_start(whs[:], wht[:])
            wos = wp.tile([H, H], MMDT, tag="wos")
            nc.sync.dma_start(wos[:], woutt[:])
            b0s = wp.tile([H, 1], F32, tag="b0s")
            nc.sync.dma_start(b0s[:], b0c[:])
            b1s = wp.tile([H, 1], F32, tag="b1s")
            nc.sync.dma_start(b1s[:], b1c2[:])

            for ch in range(NCH):
                cs = bass.ts(ch, CHUNK)

                # spread input loads across 4 DGE queues so the first-chunk
                # loads land in ~1/4 the serialized time
                c0 = io.tile([128, CHUNK], MMDT, tag="c0")
                nc.sync.dma_start(c0[:], ct[0:128, cs])
                c1 = io.tile([128, CHUNK], MMDT, tag="c1")
                nc.scalar.dma_start(c1[:], ct[128:256, cs])
                d0 = io.tile([128, CHUNK], MMDT, tag="d0")
                nc.gpsimd.dma_start(d0[:], dct[0:128, cs])
                d1 = io.tile([128, CHUNK], MMDT, tag="d1")
                nc.sync.dma_start(d1[:], dct[128:256, cs])
                hts = io.tile([128, CHUNK], MMDT, tag="hts")
                nc.scalar.dma_start(hts[:], ht[:, cs])

                # l1.T = Wxp @ csel.T + wh @ h.T   (K = 256 + 128)
                l1 = ps.tile([H, CHUNK], F32, tag="l1")
                mm(l1[:], wxp0[:], c0[:], start=True, stop=False)
                mm(l1[:], wxp1[:], c1[:], start=False, stop=False)
                mm(l1[:], whs[:], hts[:], start=False, stop=True)

                # u.T = Wxp @ dsel.T
                u = ps.tile([H, CHUNK], F32, tag="u")
                mm(u[:], wxp0[:], d0[:], start=True, stop=False)
                mm(u[:], wxp1[:], d1[:], start=False, stop=True)

                relu = tmp.tile([H, CHUNK], MMDT, tag="relu")
                nc.scalar.activation(relu[:], l1[:], AF.Relu, bias=b0s[:, 0:1])
                drelu = tmp.tile([H, CHUNK], F32, tag="drelu")
                nc.scalar.activation(drelu[:], l1[:], AF.Sigmoid, bias=b0s[:, 0:1])

                lout = ps.tile([H, CHUNK], F32, tag="lout")
                mm(lout[:], wos[:], relu[:])

                # s = sigmoid(2*(lout + b1));  dth = 1 - tanh^2 = 4*s*(1-s) = -4*q
                # with q = s^2 - s, so  dth * x == (q * -4) * x  in one DVE op.
                s = tmp.tile([H, CHUNK], F32, tag="s")
                nc.scalar.activation(s[:], lout[:], AF.Sigmoid,
                                     bias=b1s[:, 0:1], scale=2.0)
                q = tmp.tile([H, CHUNK], F32, tag="q")
                nc.vector.scalar_tensor_tensor(q[:], s[:], 1.0, s[:],
                                               OP.subtract, OP.mult)

                p1 = tmp.tile([H, CHUNK], MMDT, tag="p1")
                nc.vector.tensor_mul(p1[:], drelu[:], u[:])
                m1 = ps.tile([H, CHUNK], F32, tag="m", bufs=3)
                mm(m1[:], wos[:], p1[:])

                jx = tmp.tile([H, CHUNK], MMDT, tag="jx")
                nc.vector.scalar_tensor_tensor(jx[:], q[:], -4.0, m1[:],
                                               OP.mult, OP.mult)
                g1 = ps.tile([H, CHUNK], F32, tag="g", bufs=2)
                mm(g1[:], whs[:], jx[:])
                p2 = tmp.tile([H, CHUNK], MMDT, tag="p2")
                nc.vector.tensor_mul(p2[:], drelu[:], g1[:])
                m2 = ps.tile([H, CHUNK], F32, tag="m", bufs=3)
                mm(m2[:], wos[:], p2[:])

                jxh = tmp.tile([H, CHUNK], MMDT, tag="jxh")
                nc.vector.scalar_tensor_tensor(jxh[:], q[:], -4.0, m2[:],
                                               OP.mult, OP.mult)
                g2 = ps.tile([H, CHUNK], F32, tag="g", bufs=2)
                mm(g2[:], whs[:], jxh[:])
                p3 = tmp.tile([H, CHUNK], MMDT, tag="p3")
                nc.vector.tensor_mul(p3[:], drelu[:], g2[:])
                m3 = ps.tile([H, CHUNK], F32, tag="m", bufs=3)
                mm(m3[:], wos[:], p3[:])

                jxhh = tmp.tile([H, CHUNK], F32, tag="jxhh")
                nc.vector.scalar_tensor_tensor(jxhh[:], q[:], -4.0, m3[:],
                                               OP.mult, OP.mult)
                # final sums on the otherwise-idle GpSimd engine (SBUF-only)
                s12 = tmp.tile([H, CHUNK], F32, tag="s12")
                nc.gpsimd.tensor_add(s12[:], jx[:], jxh[:])
                outs = tmp.tile([H, CHUNK], F32, tag="outs")
                nc.gpsimd.tensor_add(outs[:], s12[:], jxhh[:])
                nc.sync.dma_start(outt[:, cs], outs[:])

    nc.compile()
    return nc


def _get_nc():
    key = USE_F32R
    if key not in _NC_CACHE:
        _NC_CACHE[key] = _build_nc(key)
    return _NC_CACHE[key]


def _prep_in_maps(t, h, coeffs, dcoeffs, tobs, wx, wh, wout, b0, b1):
    t = np.asarray(t, np.float32)
    h = np.asarray(h, np.float32)
    coeffs = np.asarray(coeffs, np.float32)
    dcoeffs = np.asarray(dcoeffs, np.float32)
    tobs = np.asarray(tobs, np.float32)
    wx = np.asarray(wx, np.float32)
    wh = np.asarray(wh, np.float32)
    wout = np.asarray(wout, np.float32)
    b0 = np.asarray(b0, np.float32)
    b1 = np.asarray(b1, np.float32)

    ts = t[0]
    idx = int(np.clip(np.searchsorted(tobs, ts, side="right") - 1, 0, NOBS - 2))
    dtv = np.float32(ts - tobs[idx])
    powers = dtv ** np.arange(4, dtype=np.float32)            # [4]
    wxp = (wx[:, :, None] * powers[None, None, :]).reshape(H, K4)

    wxpt = np.ascontiguousarray(wxp.T)                        # [256, 128]
    wht = np.ascontiguousarray(wh.T)                          # [128, 128]
    woutt = np.ascontiguousarray(wout.T)                      # [128, 128]
    b0c = np.ascontiguousarray(b0.reshape(H, 1))
    b1c2 = np.ascontiguousarray((2.0 * b1).reshape(H, 1)).astype(np.float32)

    csel = coeffs[:, idx].reshape(B, K4)                      # [B, 256]
    dsel = dcoeffs[:, idx].reshape(B, K4)

    in_maps = []
    for c in range(N_CORES):
        sl = slice(c * BS, (c + 1) * BS)
        in_maps.append({
            "ct": np.ascontiguousarray(csel[sl].T),
            "dct": np.ascontiguousarray(dsel[sl].T),
            "ht": np.ascontiguousarray(h[sl].T),
            "wxpt": wxpt,
            "wht": wht,
            "woutt": woutt,
            "b0c": b0c,
            "b1c2": b1c2,
        })
    return in_maps


def kernel(**inputs) -> np.ndarray:
    in_maps = _prep_in_maps(**inputs)
    nc = _get_nc()
    res = bass_utils.run_bass_kernel_spmd(nc, in_maps,
                                          core_ids=list(range(N_CORES)))
    out = np.empty((B, H), np.float32)
    for c in range(N_CORES):
        out[c * BS:(c + 1) * BS] = res.results[c]["outt"].T
    return out



# revision 2
# speedup vs baseline: 1.1845x; 1.1845x over previous
"""Trainium2 Bass kernel for the Neural-CDE-style cell (nn_JaCDE_88167088653055).

Math (per batch row b):
    x    = spline(coeffs, t)   xdot = spline(dcoeffs, t)
    l1   = x @ wx.T + h @ wh.T + b0
    relu = relu(l1);  drelu = sigmoid(l1)
    lout = relu @ wout.T + b1; th = tanh(lout); dth = 1 - th^2
    J(v) = dth * ((drelu * v) @ wout.T)        # action of the Jacobian factor
    jx   = J(xdot @ wx.T); jxh = J(jx @ wh.T); jxhh = J(jxh @ wh.T)
    out  = jx + jxh + jxhh

Device-side reformulation:
  * spline eval folds into the wx matmul (K = CIN*4 = 256 contraction over
    (channel, power) with host-folded weights wxp = wx (x) dt-powers).
  * everything on the PE/DVE path is bf16 (tolerance is 2e-2; bf16 keeps the
    matmuls full-rate, halves DMA bytes, and unlocks the DVE 2x mode for
    SBUF-resident elementwise ops).
  * tanh through sigmoid: s = sigmoid(2z) => dth = 1-tanh(z)^2 = -4*(s^2-s),
    so relu+sigmoid live in one ACT table set; a dummy sigmoid at t=0 makes
    walrus load that single set during the DMA prologue.
  * all five input blocks for a chunk arrive in ONE packed DMA ([128, 2560]
    bf16) - dma_start issue slices cost ~650ns each on the issuing engine.
  * final sum: t12 = jx+jxh on GpSimd off the critical path; out = t12+jxhh.

Sharding: pure data parallel - batch 8192 split as 1024 rows per core across
8 cores; small weights replicated. Activations are feature-major
([feature<=128 partitions, batch free]); every matmul is out.T = W @ act.T
with the contraction on partitions.
"""

import ml_dtypes
import numpy as np

import concourse.bass as bass
import concourse.mybir as mybir
import concourse.tile as tile
from concourse import bacc, bass_utils

N_CORES = 8
B = 8192
NOBS = 16
CIN = 64
H = 128
K4 = CIN * 4            # 256: folded (channel, power) contraction dim
BS = B // N_CORES       # 1024 batch rows per core
CHUNK = 512             # batch columns per pipeline step (one PSUM bank)
NCH = BS // CHUNK
F32 = mybir.dt.float32
BF16 = mybir.dt.bfloat16
NPBF = ml_dtypes.bfloat16

# input pack layout (bf16, per chunk): [128, 5*CHUNK]
#   [0:C)    ct rows 0:128     (csel.T upper)
#   [C:2C)   ct rows 128:256   (csel.T lower)
#   [2C:3C)  dct rows 0:128
#   [3C:4C)  dct rows 128:256
#   [4C:5C)  ht                (h.T)
PACKW = 5 * CHUNK

_NC_CACHE = {}


def _build_nc():
    AF = mybir.ActivationFunctionType
    OP = mybir.AluOpType

    nc = bacc.Bacc("TRN2", target_bir_lowering=False, debug=False,
                   enable_asserts=False, num_devices=N_CORES)

    inb = nc.dram_tensor("inb", [NCH, 128, PACKW], BF16, kind="ExternalInput")
    wpack = nc.dram_tensor("wpack", [128, 4 * H], BF16, kind="ExternalInput")
    bpack = nc.dram_tensor("bpack", [128, 2], F32, kind="ExternalInput")
    outt = nc.dram_tensor("outt", [H, BS], BF16, kind="ExternalOutput")

    def mm(out_ap, lhsT, rhs, start=True, stop=True):
        nc.tensor.matmul(out_ap, lhsT, rhs, start=start, stop=stop,
                         skip_group_check=True)

    with tile.TileContext(nc) as tc:
        with tc.tile_pool(name="w", bufs=1) as wp, \
             tc.tile_pool(name="io", bufs=2) as io, \
             tc.tile_pool(name="tmp", bufs=2) as tmp, \
             tc.tile_pool(name="ps", bufs=2, space="PSUM") as ps, \
             tc.tile_pool(name="psc", bufs=3, space="PSUM") as psc:

            ws = wp.tile([128, 4 * H], BF16, tag="ws")
            nc.scalar.dma_start(ws[:], wpack[:])
            bs_ = wp.tile([128, 2], F32, tag="bs")
            nc.scalar.dma_start(bs_[:], bpack[:])
            wxp0 = ws[:, 0:H]
            wxp1 = ws[:, H:2 * H]
            whs = ws[:, 2 * H:3 * H]
            wos = ws[:, 3 * H:4 * H]
            b0s = bs_[:, 0:1]
            b1s2 = bs_[:, 1:2]

            # dummy sigmoid: forces the (single) ACT table-set load at t=0,
            # so no table switch lands on the critical path.
            dum = wp.tile([128, 1], F32, tag="dum")
            nc.scalar.activation(dum[:], bs_[:, 0:1], AF.Sigmoid)

            for ch in range(NCH):
                cs = bass.ts(ch, CHUNK)

                it = io.tile([128, PACKW], BF16, tag="it")
                nc.sync.dma_start(it[:], inb[ch])
                c0 = it[:, 0:CHUNK]
                c1 = it[:, CHUNK:2 * CHUNK]
                d0 = it[:, 2 * CHUNK:3 * CHUNK]
                d1 = it[:, 3 * CHUNK:4 * CHUNK]
                hts = it[:, 4 * CHUNK:5 * CHUNK]

                # l1.T = Wxp @ csel.T + wh @ h.T   (K = 256 + 128)
                l1 = ps.tile([H, CHUNK], F32, tag="l1")
                mm(l1[:], wxp0, c0, start=True, stop=False)
                mm(l1[:], wxp1, c1, start=False, stop=False)
                mm(l1[:], whs, hts, start=False, stop=True)

                # u.T = Wxp @ dsel.T
                u = ps.tile([H, CHUNK], F32, tag="u")
                mm(u[:], wxp0, d0, start=True, stop=False)
                mm(u[:], wxp1, d1, start=False, stop=True)

                dr = tmp.tile([H, CHUNK], BF16, tag="dr")
                nc.scalar.activation(dr[:], l1[:], AF.Sigmoid, bias=b0s)
                relu = tmp.tile([H, CHUNK], BF16, tag="relu")
                nc.scalar.activation(relu[:], l1[:], AF.Relu, bias=b0s)

                lout = psc.tile([H, CHUNK], F32, tag="chain")
                mm(lout[:], wos, relu[:])

                # s = sigmoid(2*(lout + b1));  dth = 1-tanh^2 = -4*(s^2-s)
                s = tmp.tile([H, CHUNK], BF16, tag="s")
                nc.scalar.activation(s[:], lout[:], AF.Sigmoid,
                                     bias=b1s2, scale=2.0)
                q = tmp.tile([H, CHUNK], BF16, tag="q")
                nc.vector.scalar_tensor_tensor(q[:], s[:], 1.0, s[:],
                                               OP.subtract, OP.mult)

                p1 = tmp.tile([H, CHUNK], BF16, tag="p1")
                nc.vector.tensor_mul(p1[:], dr[:], u[:])
                m1 = psc.tile([H, CHUNK], F32, tag="chain")
                mm(m1[:], wos, p1[:])

                jx = tmp.tile([H, CHUNK], BF16, tag="jx")
                nc.vector.scalar_tensor_tensor(jx[:], m1[:], -4.0, q[:],
                                               OP.mult, OP.mult)
                g1 = psc.tile([H, CHUNK], F32, tag="chain")
                mm(g1[:], whs, jx[:])
                p2 = tmp.tile([H, CHUNK], BF16, tag="p2")
                nc.vector.tensor_mul(p2[:], dr[:], g1[:])
                m2 = psc.tile([H, CHUNK], F32, tag="chain")
                mm(m2[:], wos, p2[:])

                jxh = tmp.tile([H, CHUNK], BF16, tag="jxh")
                nc.vector.scalar_tensor_tensor(jxh[:], m2[:], -4.0, q[:],
                                               OP.mult, OP.mult)
                # t12 off the critical path on the otherwise-idle GpSimd
                t12 = tmp.tile([H, CHUNK], BF16, tag="t12")
                nc.gpsimd.tensor_add(t12[:], jx[:], jxh[:])

                g2 = psc.tile([H, CHUNK], F32, tag="chain")
                mm(g2[:], whs, jxh[:])
                p3 = tmp.tile([H, CHUNK], BF16, tag="p3")
                nc.vector.tensor_mul(p3[:], dr[:], g2[:])
                m3 = psc.tile([H, CHUNK], F32, tag="chain")
                mm(m3[:], wos, p3[:])

                jxhh = tmp.tile([H, CHUNK], BF16, tag="jxhh")
                nc.vector.scalar_tensor_tensor(jxhh[:], m3[:], -4.0, q[:],
                                               OP.mult, OP.mult)
                outs = tmp.tile([H, CHUNK], BF16, tag="outs")
                nc.gpsimd.tensor_add(outs[:], t12[:], jxhh[:])
                nc.sync.dma_start(outt[:, cs], outs[:])

    nc.compile()
    return nc


def _get_nc():
    if "nc" not in _NC_CACHE:
        _NC_CACHE["nc"] = _build_nc()
    return _NC_CACHE["nc"]


def _prep_in_maps(t, h, coeffs, dcoeffs, tobs, wx, wh, wout, b0, b1):
    t = np.asarray(t, np.float32)
    h = np.asarray(h, np.float32)
    coeffs = np.asarray(coeffs, np.float32)
    dcoeffs = np.asarray(dcoeffs, np.float32)
    tobs = np.asarray(tobs, np.float32)
    wx = np.asarray(wx, np.float32)
    wh = np.asarray(wh, np.float32)
    wout = np.asarray(wout, np.float32)
    b0 = np.asarray(b0, np.float32)
    b1 = np.asarray(b1, np.float32)

    ts = t[0]
    idx = int(np.clip(np.searchsorted(tobs, ts, side="right") - 1, 0, NOBS - 2))
    dtv = np.float32(ts - tobs[idx])
    powers = dtv ** np.arange(4, dtype=np.float32)            # [4]
    wxp = (wx[:, :, None] * powers[None, None, :]).reshape(H, K4)

    # weights pack [128, 512] bf16: wxpt upper | wxpt lower | wh.T | wout.T
    wpack = np.concatenate([
        wxp.T[0:128], wxp.T[128:256], wh.T, wout.T], axis=1).astype(NPBF)
    bpack = np.stack([b0, 2.0 * b1], axis=1).astype(np.float32)  # [128, 2]

    csel = coeffs[:, idx].reshape(B, K4).astype(NPBF)         # [B, 256]
    dsel = dcoeffs[:, idx].reshape(B, K4).astype(NPBF)
    hb = h.astype(NPBF)

    in_maps = []
    for c in range(N_CORES):
        sl = slice(c * BS, (c + 1) * BS)
        ct = np.ascontiguousarray(csel[sl].T)                 # [256, BS]
        dct = np.ascontiguousarray(dsel[sl].T)
        ht = np.ascontiguousarray(hb[sl].T)                   # [128, BS]
        inb = np.empty((NCH, 128, PACKW), NPBF)
        for ch in range(NCH):
            cs = slice(ch * CHUNK, (ch + 1) * CHUNK)
            inb[ch, :, 0:CHUNK] = ct[0:128, cs]
            inb[ch, :, CHUNK:2 * CHUNK] = ct[128:256, cs]
            inb[ch, :, 2 * CHUNK:3 * CHUNK] = dct[0:128, cs]
            inb[ch, :, 3 * CHUNK:4 * CHUNK] = dct[128:256, cs]
            inb[ch, :, 4 * CHUNK:5 * CHUNK] = ht[:, cs]
        in_maps.append({"inb": inb, "wpack": wpack, "bpack": bpack})
    return in_maps


def kernel(**inputs) -> np.ndarray:
    in_maps = _prep_in_maps(**inputs)
    nc = _get_nc()
    res = bass_utils.run_bass_kernel_spmd(nc, in_maps,
                                          core_ids=list(range(N_CORES)))
    out = np.empty((B, H), np.float32)
    for c in range(N_CORES):
        out[c * BS:(c + 1) * BS] = res.results[c]["outt"].T.astype(np.float32)
    return out
